# revision 1
# baseline (speedup 1.0000x reference)
"""DCT-II (norm='ortho') along axis 1 of x[8, 4096, 1024] on 8 NeuronCores.

Strategy: batch-parallel — core c computes the full DCT of batch c as two
half-size matmuls (Lee even/odd split):
    u[n] = x[n] + x[S-1-n],  v[n] = x[n] - x[S-1-n]   (n < S/2)
    Y[2m]   = C_e @ u        (scaled DCT-II of size S/2)
    Y[2m+1] = C_o @ v        (scaled DCT-IV of size S/2)
The row reversal of the back half is done on the tensor engine with an
anti-identity matmul (DMA access patterns reject negative strides).
Matmuls run in float32r (full-rate fp32, ~1e-4 relative accuracy).
"""

import sys

sys.path.insert(0, "/opt/trn_rl_repo")
import numpy as np

B, S, D = 8, 4096, 1024
H = S // 2
NT = H // 128  # 16 contraction tiles per branch
KT = H // 128  # 16 output tiles per branch
DC = D // 512  # 2 free-dim chunks

_cache: dict = {}


def _matrices():
    n = np.arange(H, dtype=np.float64)[None, :]
    m = np.arange(H, dtype=np.float64)[:, None]
    s2m = np.where(m == 0, np.sqrt(1.0 / S), np.sqrt(2.0 / S))
    ce = s2m * np.cos(np.pi * (2 * n + 1) * m / S)
    co = np.sqrt(2.0 / S) * np.cos(np.pi * (2 * n + 1) * (2 * m + 1) / (2 * S))

    def blocked(c):
        # per-tile lhsT layout: [nt, kt, i, j] = C[kt*128 + j, nt*128 + i]
        ct = c.T.astype(np.float32).reshape(NT, 128, KT, 128).transpose(0, 2, 1, 3)
        return np.ascontiguousarray(ct)

    j = np.eye(128, dtype=np.float32)[::-1].copy()
    return blocked(ce), blocked(co), j


def _build():
    import concourse.bacc as bacc
    import concourse.mybir as mybir
    import concourse.tile as tile

    f32 = mybir.dt.float32
    f32r = mybir.dt.float32r

    nc = bacc.Bacc("TRN2", target_bir_lowering=False, debug=False, num_devices=8)
    x_d = nc.dram_tensor("x", [S, D], f32r, kind="ExternalInput").ap()
    ce_d = nc.dram_tensor("ce", [NT, KT, 128, 128], f32r, kind="ExternalInput").ap()
    co_d = nc.dram_tensor("co", [NT, KT, 128, 128], f32r, kind="ExternalInput").ap()
    j_d = nc.dram_tensor("j", [128, 128], f32r, kind="ExternalInput").ap()
    y_d = nc.dram_tensor("y", [S, D], f32, kind="ExternalOutput").ap()
    y_v = y_d.rearrange("(k two) d -> two k d", two=2)

    with tile.TileContext(nc) as tc:
        with (
            tc.tile_pool(name="persist", bufs=1) as persist,
            tc.tile_pool(name="xin", bufs=3) as xin,
            tc.tile_pool(name="cw", bufs=8) as cw,
            tc.tile_pool(name="yout", bufs=4) as yout,
            tc.tile_pool(name="ps_rev", bufs=2, space="PSUM") as ps_rev,
            tc.tile_pool(name="ps_acc", bufs=3, space="PSUM") as ps_acc,
        ):
            jt = persist.tile([128, 128], f32r)
            nc.sync.dma_start(out=jt, in_=j_d)
            u = persist.tile([128, NT, D], f32r, tag="u")
            v = persist.tile([128, NT, D], f32r, tag="v")

            # phase 1: u = x_front + reversed(x_back), v = x_front - reversed(x_back)
            for nt in range(NT):
                xf = xin.tile([128, D], f32r, tag="xf")
                xb = xin.tile([128, D], f32r, tag="xb")
                nc.sync.dma_start(out=xf, in_=x_d[nt * 128 : (nt + 1) * 128, :])
                nc.sync.dma_start(
                    out=xb, in_=x_d[H + (NT - 1 - nt) * 128 : H + (NT - nt) * 128, :]
                )
                for dh in range(DC):
                    sl = slice(dh * 512, (dh + 1) * 512)
                    rev = ps_rev.tile([128, 512], f32)
                    nc.tensor.matmul(rev, jt, xb[:, sl], start=True, stop=True)
                    nc.vector.tensor_add(u[:, nt, sl], xf[:, sl].bitcast(f32), rev)
                    nc.vector.tensor_sub(v[:, nt, sl], xf[:, sl].bitcast(f32), rev)

            # phase 2: Y[2m+br] = C_br @ {u,v}
            for br, (mat_d, uv) in enumerate([(ce_d, u), (co_d, v)]):
                for kt in range(KT):
                    cwts = []
                    for nt2 in range(NT):
                        cwt = cw.tile([128, 128], f32r)
                        nc.sync.dma_start(out=cwt, in_=mat_d[nt2, kt])
                        cwts.append(cwt)
                    for dh in range(DC):
                        sl = slice(dh * 512, (dh + 1) * 512)
                        acc = ps_acc.tile([128, 512], f32, tag=f"acc{dh}")
                        for nt2 in range(NT):
                            nc.tensor.matmul(
                                acc,
                                cwts[nt2],
                                uv[:, nt2, sl],
                                start=(nt2 == 0),
                                stop=(nt2 == NT - 1),
                            )
                        ot = yout.tile([128, 512], f32, tag=f"ot{dh}")
                        nc.any.tensor_copy(out=ot, in_=acc)
                        nc.sync.dma_start(
                            out=y_v[br, kt * 128 : (kt + 1) * 128, sl], in_=ot
                        )
    nc.compile()
    return nc


def _get_nc():
    if "nc" not in _cache:
        _cache["nc"] = _build()
        _cache["mats"] = _matrices()
    return _cache["nc"]


def _run(x: np.ndarray, trace: bool = False):
    from concourse.bass_utils import run_bass_kernel_spmd

    nc = _get_nc()
    ce, co, j = _cache["mats"]
    x = np.ascontiguousarray(np.asarray(x, dtype=np.float32))
    in_maps = [
        {"x": np.ascontiguousarray(x[c]), "ce": ce, "co": co, "j": j}
        for c in range(B)
    ]
    res = run_bass_kernel_spmd(
        nc, in_maps, list(range(B)), trace=trace, trace_cores=[0] if trace else None
    )
    out = np.stack([res.results[c]["y"] for c in range(B)], axis=0)
    return out, res


def kernel(x: np.ndarray) -> np.ndarray:
    out, _ = _run(x, trace=False)
    return out


# revision 2
# speedup vs baseline: 1.6255x; 1.6255x over previous
"""DCT-II (norm='ortho') along axis 1 of x[8, 4096, 1024] on 8 NeuronCores.

Strategy: batch-parallel — core c computes the full DCT of batch c as two
half-size matmuls (Lee even/odd split):
    u[n] = x[n] + x[S-1-n],  v[n] = x[n] - x[S-1-n]   (n < S/2)
    Y[2m]   = C_e @ u        (scaled DCT-II of size S/2)
    Y[2m+1] = C_o @ v        (scaled DCT-IV of size S/2)
The row reversal of the back half is done on the tensor engine with an
anti-identity matmul (DMA access patterns reject negative strides).
Matmuls run in fp16 (10-bit mantissa, ~3e-4 relative error, full PE rate);
accumulation is fp32 in PSUM.
"""

import sys

sys.path.insert(0, "/opt/trn_rl_repo")
import numpy as np

B, S, D = 8, 4096, 1024
H = S // 2
NT = H // 128  # 16 contraction tiles per branch
KT = H // 128  # 16 output tiles per branch
DC = D // 512  # 2 free-dim chunks

_cache: dict = {}


def _matrices():
    n = np.arange(H, dtype=np.float64)[None, :]
    m = np.arange(H, dtype=np.float64)[:, None]
    s2m = np.where(m == 0, np.sqrt(1.0 / S), np.sqrt(2.0 / S))
    ce = s2m * np.cos(np.pi * (2 * n + 1) * m / S)
    co = np.sqrt(2.0 / S) * np.cos(np.pi * (2 * n + 1) * (2 * m + 1) / (2 * S))

    def blocked(c):
        # [kt, i, nt, j] = C[kt*128 + j, nt*128 + i]  -> ce_d[kt] is one
        # contiguous [i=128 part, nt*128 free] lhsT block for output tile kt.
        ct = c.T.astype(np.float16).reshape(NT, 128, KT, 128).transpose(2, 1, 0, 3)
        return np.ascontiguousarray(ct)

    j = np.eye(128, dtype=np.float32)[::-1].copy()
    return blocked(ce), blocked(co), j


def _build():
    import concourse.bacc as bacc
    import concourse.mybir as mybir
    import concourse.tile as tile

    f32 = mybir.dt.float32
    f32r = mybir.dt.float32r
    f16 = mybir.dt.float16

    nc = bacc.Bacc("TRN2", target_bir_lowering=False, debug=False, num_devices=8)
    x_d = nc.dram_tensor("x", [S, D], f32r, kind="ExternalInput").ap()
    ce_d = nc.dram_tensor("ce", [KT, 128, NT * 128], f16, kind="ExternalInput").ap()
    co_d = nc.dram_tensor("co", [KT, 128, NT * 128], f16, kind="ExternalInput").ap()
    j_d = nc.dram_tensor("j", [128, 128], f32r, kind="ExternalInput").ap()
    y_d = nc.dram_tensor("y", [S, D], f32, kind="ExternalOutput").ap()
    y_v = y_d.rearrange("(k two) d -> two k d", two=2)

    with tile.TileContext(nc) as tc:
        with (
            tc.tile_pool(name="persist", bufs=1) as persist,
            tc.tile_pool(name="xin", bufs=3) as xin,
            tc.tile_pool(name="cw", bufs=3) as cw,
            tc.tile_pool(name="yout", bufs=4) as yout,
            tc.tile_pool(name="ps_rev", bufs=2, space="PSUM") as ps_rev,
            tc.tile_pool(name="ps_acc", bufs=3, space="PSUM") as ps_acc,
        ):
            jt = persist.tile([128, 128], f32r)
            nc.sync.dma_start(out=jt, in_=j_d)
            u = persist.tile([128, NT, D], f16, tag="u")
            v = persist.tile([128, NT, D], f16, tag="v")

            # phase 1: u = x_front + reversed(x_back), v = x_front - reversed(x_back)
            for nt in range(NT):
                xf = xin.tile([128, D], f32r, tag="xf")
                xb = xin.tile([128, D], f32r, tag="xb")
                nc.sync.dma_start(out=xf, in_=x_d[nt * 128 : (nt + 1) * 128, :])
                nc.sync.dma_start(
                    out=xb, in_=x_d[H + (NT - 1 - nt) * 128 : H + (NT - nt) * 128, :]
                )
                for dh in range(DC):
                    sl = slice(dh * 512, (dh + 1) * 512)
                    rev = ps_rev.tile([128, 512], f32)
                    nc.tensor.matmul(rev, jt, xb[:, sl], start=True, stop=True)
                    nc.vector.tensor_add(u[:, nt, sl], xf[:, sl].bitcast(f32), rev)
                    nc.vector.tensor_sub(v[:, nt, sl], xf[:, sl].bitcast(f32), rev)

            # phase 2: Y[2m+br] = C_br @ {u,v}
            for br, (mat_d, uv) in enumerate([(ce_d, u), (co_d, v)]):
                for kt in range(KT):
                    cwt = cw.tile([128, NT * 128], f16)
                    nc.sync.dma_start(out=cwt, in_=mat_d[kt])
                    for dh in range(DC):
                        sl = slice(dh * 512, (dh + 1) * 512)
                        acc = ps_acc.tile([128, 512], f32, tag=f"acc{dh}")
                        for nt2 in range(NT):
                            nc.tensor.matmul(
                                acc,
                                cwt[:, nt2 * 128 : (nt2 + 1) * 128],
                                uv[:, nt2, sl],
                                start=(nt2 == 0),
                                stop=(nt2 == NT - 1),
                            )
                        ot = yout.tile([128, 512], f32, tag=f"ot{dh}")
                        nc.any.tensor_copy(out=ot, in_=acc)
                        nc.sync.dma_start(
                            out=y_v[br, kt * 128 : (kt + 1) * 128, sl], in_=ot
                        )
    nc.compile()
    return nc


def _get_nc():
    if "nc" not in _cache:
        _cache["nc"] = _build()
        _cache["mats"] = _matrices()
    return _cache["nc"]


def _run(x: np.ndarray, trace: bool = False):
    from concourse.bass_utils import run_bass_kernel_spmd

    nc = _get_nc()
    ce, co, j = _cache["mats"]
    x = np.ascontiguousarray(np.asarray(x, dtype=np.float32))
    in_maps = [
        {"x": np.ascontiguousarray(x[c]), "ce": ce, "co": co, "j": j}
        for c in range(B)
    ]
    res = run_bass_kernel_spmd(
        nc, in_maps, list(range(B)), trace=trace, trace_cores=[0] if trace else None
    )
    out = np.stack([res.results[c]["y"] for c in range(B)], axis=0)
    return out, res


def kernel(x: np.ndarray) -> np.ndarray:
    out, _ = _run(x, trace=False)
    return out


# revision 3
# speedup vs baseline: 2.0323x; 1.2503x over previous
"""DCT-II (norm='ortho') along axis 1 of x[8, 4096, 1024] on 8 NeuronCores.

Batch-parallel: core c computes the full DCT of batch c. Two levels of the
Lee even/odd split turn the 4096-point DCT into three matmuls:
    u[n]  = x[n] + x[S-1-n],   v[n]  = x[n] - x[S-1-n]      (n < 2048)
    u2[n] = u[n] + u[H-1-n],   v2[n] = u[n] - u[H-1-n]      (n < 1024)
    Y[2m+1] = Co2048 @ v       (scaled DCT-IV, 2048)
    Y[4p]   = Ce1024 @ u2      (scaled DCT-II, 1024)
    Y[4p+2] = Co1024 @ v2      (scaled DCT-IV, 1024)
Row reversals run on the tensor engine via an anti-identity matmul (DMA
rejects negative strides). Matmuls are fp16 (~3e-4 rel err) with fp32 PSUM
accumulation.
"""

import sys

sys.path.insert(0, "/opt/trn_rl_repo")
import numpy as np

B, S, D = 8, 4096, 1024
H = S // 2  # 2048
Q = S // 4  # 1024
NT = H // 128  # 16
NQ = Q // 128  # 8
DC = D // 512  # 2

_cache: dict = {}


def _matrices():
    s = np.full(S, np.sqrt(2.0 / S))
    s[0] = np.sqrt(1.0 / S)

    def blocked(c):
        # [kt, i, nt, j] = C[kt*128 + j, nt*128 + i]; ce_d[kt] is one
        # contiguous [i=128 part, nt*128 free] lhsT block for output tile kt.
        kt = c.shape[0] // 128
        nt = c.shape[1] // 128
        ct = c.T.astype(np.float16).reshape(nt, 128, kt, 128).transpose(2, 1, 0, 3)
        return np.ascontiguousarray(ct.reshape(kt, 128, nt * 128))

    n = np.arange(H)[None, :]
    m = np.arange(H)[:, None]
    co2048 = np.sqrt(2.0 / S) * np.cos(np.pi * (2 * n + 1) * (2 * m + 1) / (2 * S))
    n = np.arange(Q)[None, :]
    m = np.arange(Q)[:, None]
    ce1024 = s[4 * np.arange(Q)][:, None] * np.cos(np.pi * (2 * n + 1) * m / (2 * Q))
    co1024 = s[4 * np.arange(Q) + 2][:, None] * np.cos(
        np.pi * (2 * n + 1) * (2 * m + 1) / (4 * Q)
    )
    j32 = np.eye(128, dtype=np.float32)[::-1].copy()
    j16 = j32.astype(np.float16)
    return blocked(co2048), blocked(ce1024), blocked(co1024), j32, j16


def _build():
    import concourse.bacc as bacc
    import concourse.mybir as mybir
    import concourse.tile as tile

    f32 = mybir.dt.float32
    f32r = mybir.dt.float32r
    f16 = mybir.dt.float16

    nc = bacc.Bacc("TRN2", target_bir_lowering=False, debug=False, num_devices=8)
    x_d = nc.dram_tensor("x", [S, D], f32r, kind="ExternalInput").ap()
    co2048_d = nc.dram_tensor("co2048", [NT, 128, NT * 128], f16, kind="ExternalInput").ap()
    ce1024_d = nc.dram_tensor("ce1024", [NQ, 128, NQ * 128], f16, kind="ExternalInput").ap()
    co1024_d = nc.dram_tensor("co1024", [NQ, 128, NQ * 128], f16, kind="ExternalInput").ap()
    j32_d = nc.dram_tensor("j32", [128, 128], f32r, kind="ExternalInput").ap()
    j16_d = nc.dram_tensor("j16", [128, 128], f16, kind="ExternalInput").ap()
    y_d = nc.dram_tensor("y", [S, D], f32, kind="ExternalOutput").ap()
    y_v2 = y_d.rearrange("(k two) d -> two k d", two=2)
    y_v4 = y_d.rearrange("(k four) d -> four k d", four=4)

    with tile.TileContext(nc) as tc:
        with (
            tc.tile_pool(name="persist", bufs=1) as persist,
            tc.tile_pool(name="xin", bufs=3) as xin,
            tc.tile_pool(name="rev2", bufs=4) as rev2p,
            tc.tile_pool(name="cw", bufs=3) as cw,
            tc.tile_pool(name="yout", bufs=4) as yout,
            tc.tile_pool(name="ps_rev", bufs=2, space="PSUM") as ps_rev,
            tc.tile_pool(name="ps_acc", bufs=3, space="PSUM") as ps_acc,
        ):
            jt32 = persist.tile([128, 128], f32r)
            nc.sync.dma_start(out=jt32, in_=j32_d)
            jt16 = persist.tile([128, 128], f16)
            nc.sync.dma_start(out=jt16, in_=j16_d)
            u = persist.tile([128, NT, D], f16, tag="u")
            v = persist.tile([128, NT, D], f16, tag="v")
            u2 = persist.tile([128, NQ, D], f16, tag="u2")
            v2 = persist.tile([128, NQ, D], f16, tag="v2")

            # phase 1a: u/v = x_front +/- reversed(x_back)
            for nt in range(NT):
                xf = xin.tile([128, D], f32r, tag="xf")
                xb = xin.tile([128, D], f32r, tag="xb")
                nc.sync.dma_start(out=xf, in_=x_d[nt * 128 : (nt + 1) * 128, :])
                nc.sync.dma_start(
                    out=xb, in_=x_d[H + (NT - 1 - nt) * 128 : H + (NT - nt) * 128, :]
                )
                for dh in range(DC):
                    sl = slice(dh * 512, (dh + 1) * 512)
                    rev = ps_rev.tile([128, 512], f32)
                    nc.tensor.matmul(rev, jt32, xb[:, sl], start=True, stop=True)
                    nc.vector.tensor_add(u[:, nt, sl], xf[:, sl].bitcast(f32), rev)
                    nc.vector.tensor_sub(v[:, nt, sl], xf[:, sl].bitcast(f32), rev)

            # phase 1b: u2/v2 = u_front +/- reversed(u_back)
            for nq in range(NQ):
                for dh in range(DC):
                    sl = slice(dh * 512, (dh + 1) * 512)
                    rev = ps_rev.tile([128, 512], f32)
                    nc.tensor.matmul(
                        rev, jt16, u[:, NT - 1 - nq, sl], start=True, stop=True
                    )
                    rsb = rev2p.tile([128, 512], f16, tag="rsb")
                    nc.any.tensor_copy(out=rsb, in_=rev)
                    nc.vector.tensor_add(u2[:, nq, sl], u[:, nq, sl], rsb)
                    nc.vector.tensor_sub(v2[:, nq, sl], u[:, nq, sl], rsb)

            # phase 2: three matmul branches
            branches = [
                (co2048_d, v, NT, y_v2, 1),   # Y[2m+1]
                (ce1024_d, u2, NQ, y_v4, 0),  # Y[4p]
                (co1024_d, v2, NQ, y_v4, 2),  # Y[4p+2]
            ]
            for mat_d, uv, ntiles, yview, off in branches:
                for kt in range(ntiles):
                    cwt = cw.tile([128, NT * 128], f16, tag="cw")
                    nc.sync.dma_start(out=cwt[:, : ntiles * 128], in_=mat_d[kt])
                    for dh in range(DC):
                        sl = slice(dh * 512, (dh + 1) * 512)
                        acc = ps_acc.tile([128, 512], f32, tag=f"acc{dh}")
                        for nt2 in range(ntiles):
                            nc.tensor.matmul(
                                acc,
                                cwt[:, nt2 * 128 : (nt2 + 1) * 128],
                                uv[:, nt2, sl],
                                start=(nt2 == 0),
                                stop=(nt2 == ntiles - 1),
                            )
                        ot = yout.tile([128, 512], f32, tag=f"ot{dh}")
                        nc.any.tensor_copy(out=ot, in_=acc)
                        nc.sync.dma_start(
                            out=yview[off, kt * 128 : (kt + 1) * 128, sl], in_=ot
                        )
    nc.compile()
    return nc


def _get_nc():
    if "nc" not in _cache:
        _cache["nc"] = _build()
        _cache["mats"] = _matrices()
    return _cache["nc"]


def _run(x: np.ndarray, trace: bool = False):
    from concourse.bass_utils import run_bass_kernel_spmd

    nc = _get_nc()
    co2048, ce1024, co1024, j32, j16 = _cache["mats"]
    x = np.ascontiguousarray(np.asarray(x, dtype=np.float32))
    in_maps = [
        {
            "x": np.ascontiguousarray(x[c]),
            "co2048": co2048,
            "ce1024": ce1024,
            "co1024": co1024,
            "j32": j32,
            "j16": j16,
        }
        for c in range(B)
    ]
    res = run_bass_kernel_spmd(
        nc, in_maps, list(range(B)), trace=trace, trace_cores=[0] if trace else None
    )
    out = np.stack([res.results[c]["y"] for c in range(B)], axis=0)
    return out, res


def kernel(x: np.ndarray) -> np.ndarray:
    out, _ = _run(x, trace=False)
    return out


# revision 5
# speedup vs baseline: 2.0948x; 1.0307x over previous
"""DCT-II (norm='ortho') along axis 1 of x[8, 4096, 1024] on 8 NeuronCores.

Batch-parallel: core c computes the full DCT of batch c. Two levels of the
Lee even/odd split turn the 4096-point DCT into three matmuls:
    u[n]  = x[n] + x[S-1-n],   v[n]  = x[n] - x[S-1-n]      (n < 2048)
    u2[n] = u[n] + u[H-1-n],   v2[n] = u[n] - u[H-1-n]      (n < 1024)
    Y[2m+1] = Co2048 @ v       (scaled DCT-IV, 2048)
    Y[4p]   = Ce1024 @ u2      (scaled DCT-II, 1024)
    Y[4p+2] = Co1024 @ v2      (scaled DCT-IV, 1024)
Row reversals run on the tensor engine via an anti-identity matmul (DMA
rejects negative strides). Matmuls are fp16 (~3e-4 rel err) with fp32 PSUM
accumulation.
"""

import sys

sys.path.insert(0, "/opt/trn_rl_repo")
import numpy as np

B, S, D = 8, 4096, 1024
H = S // 2  # 2048
Q = S // 4  # 1024
NT = H // 128  # 16
NQ = Q // 128  # 8
DC = D // 512  # 2

_cache: dict = {}


def _matrices():
    s = np.full(S, np.sqrt(2.0 / S))
    s[0] = np.sqrt(1.0 / S)

    def blocked(c):
        # [kt, i, nt, j] = C[kt*128 + j, nt*128 + i]; ce_d[kt] is one
        # contiguous [i=128 part, nt*128 free] lhsT block for output tile kt.
        kt = c.shape[0] // 128
        nt = c.shape[1] // 128
        ct = c.T.astype(np.float16).reshape(nt, 128, kt, 128).transpose(2, 1, 0, 3)
        return np.ascontiguousarray(ct.reshape(kt, 128, nt * 128))

    n = np.arange(H)[None, :]
    m = np.arange(H)[:, None]
    co2048 = np.sqrt(2.0 / S) * np.cos(np.pi * (2 * n + 1) * (2 * m + 1) / (2 * S))
    n = np.arange(Q)[None, :]
    m = np.arange(Q)[:, None]
    ce1024 = s[4 * np.arange(Q)][:, None] * np.cos(np.pi * (2 * n + 1) * m / (2 * Q))
    co1024 = s[4 * np.arange(Q) + 2][:, None] * np.cos(
        np.pi * (2 * n + 1) * (2 * m + 1) / (4 * Q)
    )
    j32 = np.eye(128, dtype=np.float32)[::-1].copy()
    j16 = j32.astype(np.float16)
    return blocked(co2048), blocked(ce1024), blocked(co1024), j32, j16


def _build():
    import concourse.bacc as bacc
    import concourse.mybir as mybir
    import concourse.tile as tile

    f32 = mybir.dt.float32
    f32r = mybir.dt.float32r
    f16 = mybir.dt.float16

    nc = bacc.Bacc("TRN2", target_bir_lowering=False, debug=False, num_devices=8)
    x_d = nc.dram_tensor("x", [S, D], f32r, kind="ExternalInput").ap()
    co2048_d = nc.dram_tensor("co2048", [NT, 128, NT * 128], f16, kind="ExternalInput").ap()
    ce1024_d = nc.dram_tensor("ce1024", [NQ, 128, NQ * 128], f16, kind="ExternalInput").ap()
    co1024_d = nc.dram_tensor("co1024", [NQ, 128, NQ * 128], f16, kind="ExternalInput").ap()
    j32_d = nc.dram_tensor("j32", [128, 128], f32r, kind="ExternalInput").ap()
    j16_d = nc.dram_tensor("j16", [128, 128], f16, kind="ExternalInput").ap()
    y_d = nc.dram_tensor("y", [S, D], f32, kind="ExternalOutput").ap()
    y_v2 = y_d.rearrange("(k two) d -> two k d", two=2)
    y_v4 = y_d.rearrange("(k four) d -> four k d", four=4)

    with tile.TileContext(nc) as tc:
        with (
            tc.tile_pool(name="persist", bufs=1) as persist,
            tc.tile_pool(name="xin", bufs=3) as xin,
            tc.tile_pool(name="rev2", bufs=4) as rev2p,
            tc.tile_pool(name="cw", bufs=4) as cw,
            tc.tile_pool(name="yout", bufs=4) as yout,
            tc.tile_pool(name="ps_rev", bufs=2, space="PSUM") as ps_rev,
            tc.tile_pool(name="ps_acc", bufs=3, space="PSUM") as ps_acc,
        ):
            jt32 = persist.tile([128, 128], f32r)
            nc.sync.dma_start(out=jt32, in_=j32_d)
            jt16 = persist.tile([128, 128], f16)
            nc.sync.dma_start(out=jt16, in_=j16_d)
            u = persist.tile([128, NT, D], f16, tag="u")
            v = persist.tile([128, NT, D], f16, tag="v")
            u2 = persist.tile([128, NQ, D], f16, tag="u2")
            v2 = persist.tile([128, NQ, D], f16, tag="v2")

            # phase 1a: u/v = x_front +/- reversed(x_back)
            for nt in range(NT):
                xf = xin.tile([128, D], f32r, tag="xf")
                xb = xin.tile([128, D], f32r, tag="xb")
                nc.sync.dma_start(out=xf, in_=x_d[nt * 128 : (nt + 1) * 128, :])
                nc.sync.dma_start(
                    out=xb, in_=x_d[H + (NT - 1 - nt) * 128 : H + (NT - nt) * 128, :]
                )
                for dh in range(DC):
                    sl = slice(dh * 512, (dh + 1) * 512)
                    rev = ps_rev.tile([128, 512], f32)
                    nc.tensor.matmul(rev, jt32, xb[:, sl], start=True, stop=True)
                    nc.vector.tensor_add(u[:, nt, sl], xf[:, sl].bitcast(f32), rev)
                    nc.vector.tensor_sub(v[:, nt, sl], xf[:, sl].bitcast(f32), rev)

            # phase 1b: u2/v2 = u_front +/- reversed(u_back)
            for nq in range(NQ):
                for dh in range(DC):
                    sl = slice(dh * 512, (dh + 1) * 512)
                    rev = ps_rev.tile([128, 512], f32)
                    nc.tensor.matmul(
                        rev, jt16, u[:, NT - 1 - nq, sl], start=True, stop=True
                    )
                    rsb = rev2p.tile([128, 512], f16, tag="rsb")
                    nc.any.tensor_copy(out=rsb, in_=rev)
                    nc.vector.tensor_add(u2[:, nq, sl], u[:, nq, sl], rsb)
                    nc.vector.tensor_sub(v2[:, nq, sl], u[:, nq, sl], rsb)

            # phase 2: three matmul branches
            branches = [
                (co2048_d, v, NT, y_v2, 1),   # Y[2m+1]
                (ce1024_d, u2, NQ, y_v4, 0),  # Y[4p]
                (co1024_d, v2, NQ, y_v4, 2),  # Y[4p+2]
            ]
            for mat_d, uv, ntiles, yview, off in branches:
                for kt in range(ntiles):
                    cwt = cw.tile([128, NT * 128], f16, tag="cw")
                    nc.gpsimd.dma_start(out=cwt[:, : ntiles * 128], in_=mat_d[kt])
                    for dh in range(DC):
                        sl = slice(dh * 512, (dh + 1) * 512)
                        acc = ps_acc.tile([128, 512], f32, tag=f"acc{dh}")
                        for nt2 in range(ntiles):
                            nc.tensor.matmul(
                                acc,
                                cwt[:, nt2 * 128 : (nt2 + 1) * 128],
                                uv[:, nt2, sl],
                                start=(nt2 == 0),
                                stop=(nt2 == ntiles - 1),
                            )
                        ot = yout.tile([128, 512], f32, tag=f"ot{dh}")
                        nc.any.tensor_copy(out=ot, in_=acc)
                        nc.scalar.dma_start(
                            out=yview[off, kt * 128 : (kt + 1) * 128, sl], in_=ot
                        )
    nc.compile()
    return nc


def _get_nc():
    if "nc" not in _cache:
        _cache["nc"] = _build()
        _cache["mats"] = _matrices()
    return _cache["nc"]


def _run(x: np.ndarray, trace: bool = False):
    from concourse.bass_utils import run_bass_kernel_spmd

    nc = _get_nc()
    co2048, ce1024, co1024, j32, j16 = _cache["mats"]
    x = np.ascontiguousarray(np.asarray(x, dtype=np.float32))
    in_maps = [
        {
            "x": np.ascontiguousarray(x[c]),
            "co2048": co2048,
            "ce1024": ce1024,
            "co1024": co1024,
            "j32": j32,
            "j16": j16,
        }
        for c in range(B)
    ]
    res = run_bass_kernel_spmd(
        nc, in_maps, list(range(B)), trace=trace, trace_cores=[0] if trace else None
    )
    out = np.stack([res.results[c]["y"] for c in range(B)], axis=0)
    return out, res


def kernel(x: np.ndarray) -> np.ndarray:
    out, _ = _run(x, trace=False)
    return out
